# revision 30
# baseline (speedup 1.0000x reference)
"""Binarized conv1d (k=7, pad=3 with -1.0) + maxpool(2) + PReLU + BatchNorm1d
(training stats) fused Trainium2 kernel, data-parallel over batch N across 8
NeuronCores with an on-chip AllReduce for the BN batch statistics.

Contract: kernel(**inputs) takes the FULL inputs from setup_inputs() and
returns the FULL [128, 128, 2048] float32 output.

Algorithm per core (16 of the 128 batches), final (fp8 DoubleRow rewrite):
  - activations encoded h' = (x>=0) - 0.5 in {+-0.5} fp8e4 (pad -0.5), so the
    +-1 binarized conv is exactly 2*conv(h', sign(W)); the 2x folds into the
    BN statistics scalar math. One pair-wide vector-engine tensor_scalar
    (f32 single-src, 2-port DVE mode) binarizes two batches at once into U;
    DMA copies then place the direct and shifted-by-2 partition halves.
    GPSIMD is kept off the compute path entirely: concurrent GPSIMD
    instructions block DVE accum_out ops for their full duration (measured).
  - the conv+maxpool is computed as two stride-2 convs, E (even positions)
    and O (odd positions, tap weights shifted by one); since prelu is
    monotone, maxpool+PReLU = max(prelu(E), prelu(O)): the scalar engine
    applies one Prelu per conv straight out of a 4-bank PSUM tile, and one
    vector STT takes the max, with accum_out emitting the per-channel
    sum(y') for free. sum(y'^2) via one more vector STT per batch.
    The scalar-engine queue carries nothing but Prelu ops during the conv
    phase - anything else head-of-line blocks the PSUM drain and stalls the
    PE (measured with Square and Sign ops interleaved).
  - conv matmuls use fp8e4 DoubleRow perf mode: contraction 256 = 64 ch x 4
    taps per matmul (partition (c,g) holds h' shifted by 2g; the in-pair tap
    comes from the DoubleRow pair dim with stride 1, output stride 2). Per
    512-wide psum bank: 2 accumulated matmuls instead of 4 bf16 ones; MMs
    stream at the 215ns/512-col cadence with LDWEIGHTS hidden. Even/odd
    batches swap the shifted/direct partition halves so the h' write stays
    lane-aligned; both weight layouts precomputed on the host (the x0.5
    sets for a +-1 encoding variant are shipped but unused). O-convs are
    issued first so the scalar engine drains PSUM while E still computes.
  - BN stats (sum, sumsq) all-reduced across the 8 cores with a SINGLE
    remote-DMA broadcast to all 8 destinations (fanned over all 16 DMA
    engines in parallel; the landing slot is indexed by the sender's
    runtime partition_id so every slot is single-writer). This replaces 7
    serialized per-peer sends whose CC transfers cost ~55us. Each core then
    computes scale/shift (Newton-corrected sqrt) and streams s*y+t back to
    HBM on both HWDGE queues, norm on the scalar engine under the
    DMA-bound (~47us) store tail.
"""

import uuid

import numpy as np
import ml_dtypes
import jax

# The jax persistent compilation cache mis-keys bass_exec custom-call
# executables (the embedded NEFF differs while the cache key does not),
# which can hand back a stale executable and wedge the device. Disable it.
jax.config.update("jax_enable_compilation_cache", False)

import concourse.bacc as bacc
import concourse.mybir as mybir
import concourse.tile as tile
from concourse.bass_utils import run_bass_kernel_spmd

AF = mybir.ActivationFunctionType
ALU = mybir.AluOpType

N_CORES = 8
N = 128            # total batch
NB = N // N_CORES  # batches per core = 16
CI = 64            # in channels
CO = 128           # out channels
L = 4096           # input length
LO = L // 2        # pooled output length = 2048
K = 7              # kernel taps
TW = L + 4         # h' tile width: pads 0:3 / L+3, data 3:L+3
PAD_VAL = -1.0
EPS = 1e-5
M_GLOBAL = float(N * LO)  # BN reduction count per channel

FP8 = mybir.dt.float8e4
FP8NP = mybir.dt.np(FP8)

T_BUFS = 4   # must be even: pad layout alternates with batch parity
_AR_STATE = {}
XT_BUFS = 2


def _build(alpha: float):
    nc = bacc.Bacc("TRN2", target_bir_lowering=False, debug=False,
                   num_devices=N_CORES)

    xs = nc.dram_tensor("xs", [NB * CI, L], mybir.dt.float32, kind="ExternalInput")
    wts = nc.dram_tensor("wts", [128, 16 * 256], FP8, kind="ExternalInput")
    gb = nc.dram_tensor("gb", [128, 2], mybir.dt.float32, kind="ExternalInput")
    out = nc.dram_tensor("out", [NB * CO, LO], mybir.dt.float32, kind="ExternalOutput")

    with tile.TileContext(nc) as tc:
        with (
            tc.tile_pool(name="wp", bufs=1) as wp,
            tc.tile_pool(name="xp", bufs=XT_BUFS) as xp,
            tc.tile_pool(name="tp", bufs=T_BUFS) as tp,
            tc.tile_pool(name="pp", bufs=1, space="PSUM") as pp,
            tc.tile_pool(name="ep", bufs=4) as ep,
            tc.tile_pool(name="fp", bufs=6) as fp,
            tc.tile_pool(name="yp", bufs=NB) as yp,
            tc.tile_pool(name="sp", bufs=1) as sp,
            tc.tile_pool(name="up", bufs=3) as up,
            tc.tile_pool(name="op", bufs=2) as op_pool,
        ):
            # weights/params on the scalar HWDGE queue so the first x load
            # owns the sync queue from t=0
            wt = wp.tile([128, 16 * 256], FP8)
            nc.scalar.dma_start(wt[:], wts[:])
            gbt = wp.tile([128, 2], mybir.dt.float32)
            nc.scalar.dma_start(gbt[:], gb[:])

            # trigger the Prelu ACT table load during the DMA ramp (the set
            # also serves Relu)
            warm = wp.tile([128, 1], mybir.dt.float32)
            nc.vector.memset(warm[:], 1.0)
            nc.scalar.activation(warm[:], warm[:], AF.Prelu, alpha=alpha)

            # ---- AllReduce setup, off the critical path ----
            # Pay the gpsimd ext-isa IRAM load (~6us) and descriptor
            # generation for the stats broadcast now, while the conv runs.
            # The broadcast DMA itself is only triggered after the stats are
            # final (the descriptor holds addresses, not data).
            loc = sp.tile([128, 2], mybir.dt.float32, name="loc", tag="loc")
            xbuf = sp.tile([128, 16], mybir.dt.float32, name="xbuf", tag="xbuf")
            rsem = nc.alloc_semaphore("ar_remote")
            psem = nc.alloc_semaphore("ar_prep")
            lsem = nc.alloc_semaphore("ar_local")
            _AR_STATE.update(loc=loc, xbuf=xbuf, rsem=rsem,
                             psem=psem, lsem=lsem)

            _build_pass(nc, tc, xs, out, wt, gbt, alpha,
                        xp, tp, pp, ep, fp, yp, sp, up, op_pool)

    nc.compile()
    nc.m.name = f"bk{uuid.uuid4().hex[:10]}"
    return nc


def _conv_mms(nc, wt, T, ps_full, conv_i, parity, enc):
    """Issue the 8 DoubleRow matmuls of one conv (O or E) for one batch:
    j in {0,1} accumulated over 4 psum banks of a [128, 2048] tile."""
    for j in range(2):
        off = (enc * 8 + parity * 4 + conv_i * 2 + j) * 256
        lhsT = wt[:, off:off + 256].rearrange("p (two m) -> p two m", two=2)
        for q in range(4):
            rhs = T[:, 4 * j + 1024 * q: 4 * j + 1024 * q + 1024].rearrange(
                "p (n two) -> p two n", two=2)
            nc.tensor.matmul(
                ps_full[:, q * 512:(q + 1) * 512], lhsT, rhs,
                start=(j == 0), stop=(j == 1),
                perf_mode=mybir.MatmulPerfMode.DoubleRow)


def _build_pass(nc, tc, xs, out, wt, gbt, alpha,
                xp, tp, pp, ep, fp, yp, sp, up, op_pool):
    # stats: cols 0:16 per-batch sum(y') (STT max accum), 16:32 sum(y'^2)
    stats = sp.tile([128, 32], mybir.dt.float32, name="stats", tag="stats")

    y_tiles = []

    def produce_pair(bp):
        """Load x for batch pair bp and build both h' tiles (fp8, shifted).
        One pair-wide is_ge (both batches in one DVE op) lands in U; DMA
        copies then place the direct and shifted-by-2 halves per batch."""
        xt = xp.tile([128, L], mybir.dt.float32, name=f"xt{bp}", tag="xt")
        nc.sync.dma_start(xt[:], xs[bp * 128:(bp + 1) * 128, :])
        U = up.tile([128, L], FP8, name=f"U{bp}", tag="U")
        enc = 0
        if True:
            # h' = (x>=0) - 0.5 on the vector engine
            nc.vector.tensor_scalar(U[:], xt[:], 0.0, 0.5,
                                    op0=ALU.is_ge, op1=ALU.subtract)
        else:
            # sign(x) = +-1 on the scalar engine (paired with 0.5x weights);
            # odd pairs always land in T buffers 2,3 so the -1.0 pads stay
            # consistent across buffer reuse
            nc.scalar.activation(U[:], xt[:], AF.Sign)
        pad = -0.5 if enc == 0 else -1.0
        Ts = []
        for sub in range(2):
            b = 2 * bp + sub
            lo, hi = (0, 64) if sub == 0 else (64, 128)
            ol, oh = (64, 128) if sub == 0 else (0, 64)
            T = tp.tile([128, TW], FP8, name=f"T{b}", tag="T")
            if b < T_BUFS:
                # first use of each of the 4 T buffers: set the pads (enc is
                # stable per buffer: even pairs -> bufs 0/1, odd -> 2/3)
                nc.vector.memset(T[lo:hi, 0:3], pad)
                nc.vector.memset(T[lo:hi, L + 3:TW], pad)
                nc.vector.memset(T[ol:oh, 0:1], pad)
                nc.vector.memset(T[ol:oh, L + 1:TW], pad)
            # direct half on sync, shifted-by-2 half on scalar (pair 0:
            # both on sync, clear of the weight DMAs on the scalar queue)
            ceng = nc.sync if bp == 0 else nc.scalar
            nc.sync.dma_start(T[lo:hi, 3:L + 3], U[lo:hi, :])
            ceng.dma_start(T[ol:oh, 1:L + 1], U[lo:hi, :])
            Ts.append(T)
        return Ts

    def conv_pair(bp, Ts):
        enc = 0
        for sub in range(2):
            b = 2 * bp + sub
            T = Ts[sub]
            o_full = pp.tile([128, LO], mybir.dt.float32, name=f"o{b}", tag="o")
            e_full = pp.tile([128, LO], mybir.dt.float32, name=f"e{b}", tag="e")

            # O first: its prelu frees the banks while E still computes
            _conv_mms(nc, wt, T, o_full, 0, sub, enc)
            _conv_mms(nc, wt, T, e_full, 1, sub, enc)

            po = fp.tile([128, LO], mybir.dt.float16, name=f"po{b}", tag="po")
            pe = ep.tile([128, LO], mybir.dt.float16, name=f"pe{b}", tag="pe")
            nc.scalar.activation(po[:], o_full[:], AF.Prelu, alpha=alpha)
            nc.scalar.activation(pe[:], e_full[:], AF.Prelu, alpha=alpha)

            yt = yp.tile([128, LO], mybir.dt.float16, name=f"yt{b}", tag="yt")
            y_tiles.append(yt)
            # tensor_tensor has a 2x uop (STT is 1x-only); the per-channel
            # sums come from in-place 4x tensor_scalar copies with accum_out
            nc.vector.scalar_tensor_tensor(
                yt[:], pe[:], 0.0, po[:], op0=ALU.bypass, op1=ALU.max,
                accum_out=stats[:, b:b + 1])
            sq = fp.tile([128, LO], mybir.dt.float16, name=f"sq{b}", tag="po")
            nc.vector.scalar_tensor_tensor(
                sq[:], yt[:], 1.0, yt[:], op0=ALU.mult, op1=ALU.mult,
                accum_out=stats[:, 16 + b:17 + b])

    Ts_ahead = produce_pair(0)
    for bp in range(NB // 2):
        Ts_cur = Ts_ahead
        if bp + 1 < NB // 2:
            Ts_ahead = produce_pair(bp + 1)
        conv_pair(bp, Ts_cur)

    # ---- local partial stats -> remote-DMA all-reduce -> scale/shift ----
    # One-shot broadcast: every core sends its [128,2] partial (sum, sumsq)
    # to ALL 8 cores (itself included) in a single remote_dma_broadcast --
    # the ucode fans the 8 destinations across all 16 DMA engines in
    # parallel, vs ~7 serialized CC transfers for per-peer sends. The
    # landing slot is indexed by the sender id (runtime partition_id offset)
    # so every slot is single-writer. The descriptor was pre-generated at
    # kernel start (see above); here we only barrier + trigger + wait.
    loc = _AR_STATE["loc"]
    nc.vector.tensor_reduce(loc[:, 0:1], stats[:, 0:16],
                            axis=mybir.AxisListType.X, op=ALU.add)
    nc.vector.tensor_reduce(loc[:, 1:2], stats[:, 16:32],
                            axis=mybir.AxisListType.X, op=ALU.add)

    xbuf = _AR_STATE["xbuf"]
    g = sp.tile([128, 2], mybir.dt.float32, name="g", tag="g")

    rsem = _AR_STATE["rsem"]
    psem = _AR_STATE["psem"]
    lsem = _AR_STATE["lsem"]
    with tc.tile_critical(no_gpsimd_drain=True):
        nc.gpsimd.bir_kernel_barrier_wait([list(range(N_CORES))])
        # slot = this core's id, so every slot is single-writer
        slot = xbuf[:, 0:2].copy()
        slot.offset = nc.gpsimd.partition_id() * 2
        nc.gpsimd.remote_dma_broadcast(
            slot, loc[:, 0:2], rsem, lsem,
            rdests=[(0, k) for k in range(N_CORES)],
        ).then_inc(psem, 1)
        nc.gpsimd.wait_ge(psem, 1)
        nc.gpsimd.trigger_dma(count=None)
        # 8 arriving broadcasts (incl self) x (16//8)=2 incs each
        nc.vector.wait_ge(rsem, 16)
        nc.vector.tensor_reduce(
            g[:, 0:1], xbuf.rearrange("p (s two) -> p two s", two=2)[:, 0:1, :],
            axis=mybir.AxisListType.X, op=ALU.add)
        nc.vector.tensor_reduce(
            g[:, 1:2], xbuf.rearrange("p (s two) -> p two s", two=2)[:, 1:2, :],
            axis=mybir.AxisListType.X, op=ALU.add)

    # mean/var/scale/shift, all [128,1] f32. y = 2*y' so:
    #   mean = 2*S1/M ; E[y^2] = 4*S2/M ; out = (2*s)*y' + (beta - s*mean)
    v = sp.tile([128, 8], mybir.dt.float32, name="v", tag="v")
    mean, msq_eps, vareps, std, rec, t1, s2_col, t_col = (
        v[:, i:i + 1] for i in range(8))
    nc.vector.tensor_scalar(mean, g[:, 0:1], 2.0 / M_GLOBAL, None, op0=ALU.mult)
    # msq_eps = mean^2 - eps
    nc.vector.tensor_scalar(msq_eps, mean, mean, EPS, op0=ALU.mult, op1=ALU.subtract)
    # vareps = 4*ssq/M - (mean^2 - eps) = var + eps
    nc.vector.scalar_tensor_tensor(
        vareps, g[:, 1:2], 4.0 / M_GLOBAL, msq_eps,
        op0=ALU.mult, op1=ALU.subtract)
    nc.scalar.activation(std, vareps, AF.Sqrt)
    # one Newton step: std = 0.5*(std + vareps/std)
    nc.vector.reciprocal(rec, std)
    # t1 = 0.5 * vareps / std
    nc.vector.tensor_scalar(t1, rec, vareps, 0.5, op0=ALU.mult, op1=ALU.mult)
    nc.vector.scalar_tensor_tensor(std, std, 0.5, t1,
                                   op0=ALU.mult, op1=ALU.add)
    nc.vector.reciprocal(rec, std)
    # s2 = 2 * gamma / std
    nc.vector.tensor_scalar(s2_col, rec, gbt[:, 0:1], 2.0,
                            op0=ALU.mult, op1=ALU.mult)
    # t = beta - s2*mean/2
    nc.vector.tensor_scalar(t1, mean, -0.5, None, op0=ALU.mult)
    nc.vector.scalar_tensor_tensor(
        t_col, s2_col, t1, gbt[:, 1:2], op0=ALU.mult, op1=ALU.add)

    # ---- pass 2: normalize + store. Two batches per output tile (fewer,
    # bigger DMAs); out-DMAs alternate across both HWDGE queues ----
    for bp in range(NB // 2):
        ot = op_pool.tile([128, 2 * LO], mybir.dt.float32, name=f"ot{bp}", tag="ot")
        for sub in range(2):
            nc.scalar.activation(
                ot[:, sub * LO:(sub + 1) * LO], y_tiles[2 * bp + sub][:],
                AF.Identity, bias=t_col, scale=s2_col)
        eng = nc.sync if bp % 2 == 0 else nc.scalar
        eng.dma_start(
            out.rearrange("(a p) l -> p a l", p=128)[:, 2 * bp:2 * bp + 2, :],
            ot.rearrange("p (a l) -> p a l", a=2))


def _swinterleave(wt):
    """Repack [128, nsets, 2, 128] lhsT sets into the DoubleRowSwInterleave
    layout: per partition row [A127, B127, A126, B126, ..., A0, B0]."""
    k, ns, two, m = wt.shape
    out = np.zeros((k, ns, 2 * m), wt.dtype)
    for c in range(m):
        out[:, :, 2 * c] = wt[:, :, 0, m - 1 - c]
        out[:, :, 2 * c + 1] = wt[:, :, 1, m - 1 - c]
    return out.reshape(k, ns, 2, m)


def _prep_weights(W: np.ndarray) -> np.ndarray:
    """Host-side: pack the 8 DoubleRow lhsT matrices [128, 2, 128] fp8:
    (parity even/odd) x (conv O/E) x (j 0/1). Partition k=(c,g): channel
    c=k%64, shift s(k) (0/2 direct/shifted, swapped for odd parity); pair
    element i is tap 4j + i + s(k)."""
    bw = np.sign(W).astype(np.float32)          # [CO, CI, K]
    wh = np.zeros((CO, CI, 8), np.float32)
    wh[:, :, :K] = bw
    wo = np.zeros((CO, CI, 8), np.float32)      # O-conv taps: w[t-1]
    wo[:, :, 1:8] = wh[:, :, 0:7]

    wt = np.zeros((128, 8, 2, 128), np.float32)  # [k, set, i, o]
    ks = np.arange(128)
    cs = ks % 64
    for parity in range(2):
        s_of_k = np.where(ks < 64, 0, 2) if parity == 0 else \
            np.where(ks < 64, 2, 0)
        for conv_i, wsrc in enumerate([wo, wh]):
            for j in range(2):
                si = (parity * 2 + conv_i) * 2 + j
                for i in range(2):
                    t = 4 * j + i + s_of_k          # [128]
                    wt[ks, si, i, :] = wsrc[:, cs, t].T
    # enc 1 (activations +-1 via ACT Sign) uses the same weights halved,
    # so both encodings produce identical conv values
    full = np.concatenate([wt, 0.5 * wt], axis=1)  # [128, 16, 2, 128]
    return full.reshape(128, 16 * 256).astype(FP8NP)


_NC_CACHE = {}


def kernel(x, W, prelu_w, gamma, beta):
    x = np.asarray(x)
    W = np.asarray(W)
    alpha = float(np.asarray(prelu_w).reshape(-1)[0])
    gamma = np.asarray(gamma, dtype=np.float32)
    beta = np.asarray(beta, dtype=np.float32)

    assert x.shape == (N, CI, L), x.shape
    wts = _prep_weights(W)
    gb = np.stack([gamma, beta], axis=1).astype(np.float32)

    key = alpha
    if key not in _NC_CACHE:
        _NC_CACHE[key] = _build(alpha)
    nc = _NC_CACHE[key]

    in_maps = []
    for c in range(N_CORES):
        shard = np.ascontiguousarray(
            x[c * NB:(c + 1) * NB].reshape(NB * CI, L), dtype=np.float32)
        in_maps.append({"xs": shard, "wts": wts, "gb": gb})

    res = run_bass_kernel_spmd(nc, in_maps, core_ids=list(range(N_CORES)))
    outs = [res.results[c]["out"].reshape(NB, CO, LO) for c in range(N_CORES)]
    return np.concatenate(outs, axis=0)


# revision 31
# speedup vs baseline: 1.0619x; 1.0619x over previous
"""Binarized conv1d (k=7, pad=3 with -1.0) + maxpool(2) + PReLU + BatchNorm1d
(training stats) fused Trainium2 kernel, data-parallel over batch N across 8
NeuronCores with an on-chip AllReduce for the BN batch statistics.

Contract: kernel(**inputs) takes the FULL inputs from setup_inputs() and
returns the FULL [128, 128, 2048] float32 output.

Algorithm per core (16 of the 128 batches), final (fp8 DoubleRow rewrite):
  - activations encoded h' = (x>=0) - 0.5 in {+-0.5} fp8e4 (pad -0.5), so the
    +-1 binarized conv is exactly 2*conv(h', sign(W)); the 2x folds into the
    BN statistics scalar math. One pair-wide vector-engine tensor_scalar
    (f32 single-src, 2-port DVE mode) binarizes two batches at once into U;
    DMA copies then place the direct and shifted-by-2 partition halves.
    GPSIMD is kept off the compute path entirely: concurrent GPSIMD
    instructions block DVE accum_out ops for their full duration (measured).
  - the conv+maxpool is computed as two stride-2 convs, E (even positions)
    and O (odd positions, tap weights shifted by one); since prelu is
    monotone, maxpool+PReLU = max(prelu(E), prelu(O)): the scalar engine
    applies one Prelu per conv straight out of a 4-bank PSUM tile, and one
    vector STT takes the max, with accum_out emitting the per-channel
    sum(y') for free. sum(y'^2) via one more vector STT per batch.
    The scalar-engine queue carries nothing but Prelu ops during the conv
    phase - anything else head-of-line blocks the PSUM drain and stalls the
    PE (measured with Square and Sign ops interleaved).
  - conv matmuls use fp8e4 DoubleRow perf mode: contraction 256 = 64 ch x 4
    taps per matmul (partition (c,g) holds h' shifted by 2g; the in-pair tap
    comes from the DoubleRow pair dim with stride 1, output stride 2). Per
    512-wide psum bank: 2 accumulated matmuls instead of 4 bf16 ones; MMs
    stream at the 215ns/512-col cadence with LDWEIGHTS hidden. Even/odd
    batches swap the shifted/direct partition halves so the h' write stays
    lane-aligned; both weight layouts precomputed on the host (the x0.5
    sets for a +-1 encoding variant are shipped but unused). O-convs are
    issued first so the scalar engine drains PSUM while E still computes.
  - BN stats (sum, sumsq) all-reduced across the 8 cores with a SINGLE
    remote-DMA broadcast to all 8 destinations (fanned over all 16 DMA
    engines in parallel; the landing slot is indexed by the sender's
    runtime partition_id so every slot is single-writer). This replaces 7
    serialized per-peer sends whose CC transfers cost ~55us. Each core then
    computes scale/shift (Newton-corrected sqrt) and streams s*y+t back to
    HBM on both HWDGE queues, norm on the scalar engine under the
    DMA-bound (~47us) store tail.
"""

import uuid

import numpy as np
import ml_dtypes
import jax

# The jax persistent compilation cache mis-keys bass_exec custom-call
# executables (the embedded NEFF differs while the cache key does not),
# which can hand back a stale executable and wedge the device. Disable it.
jax.config.update("jax_enable_compilation_cache", False)

import concourse.bacc as bacc
import concourse.mybir as mybir
import concourse.tile as tile
from concourse.bass_utils import run_bass_kernel_spmd

AF = mybir.ActivationFunctionType
ALU = mybir.AluOpType

N_CORES = 8
N = 128            # total batch
NB = N // N_CORES  # batches per core = 16
CI = 64            # in channels
CO = 128           # out channels
L = 4096           # input length
LO = L // 2        # pooled output length = 2048
K = 7              # kernel taps
TW = L + 4         # h' tile width: pads 0:3 / L+3, data 3:L+3
PAD_VAL = -1.0
EPS = 1e-5
M_GLOBAL = float(N * LO)  # BN reduction count per channel

FP8 = mybir.dt.float8e4
FP8NP = mybir.dt.np(FP8)

T_BUFS = 4   # must be even: pad layout alternates with batch parity
_AR_STATE = {}
XT_BUFS = 2


def _build(alpha: float):
    nc = bacc.Bacc("TRN2", target_bir_lowering=False, debug=False,
                   num_devices=N_CORES)

    xs = nc.dram_tensor("xs", [NB * CI, L], mybir.dt.float32, kind="ExternalInput")
    wts = nc.dram_tensor("wts", [128, 16 * 256], FP8, kind="ExternalInput")
    gb = nc.dram_tensor("gb", [128, 2], mybir.dt.float32, kind="ExternalInput")
    out = nc.dram_tensor("out", [NB * CO, LO], mybir.dt.float32, kind="ExternalOutput")

    with tile.TileContext(nc) as tc:
        with (
            tc.tile_pool(name="wp", bufs=1) as wp,
            tc.tile_pool(name="xp", bufs=XT_BUFS) as xp,
            tc.tile_pool(name="tp", bufs=T_BUFS) as tp,
            tc.tile_pool(name="pp", bufs=2, space="PSUM") as pp,
            tc.tile_pool(name="ep", bufs=4) as ep,
            tc.tile_pool(name="fp", bufs=6) as fp,
            tc.tile_pool(name="yp", bufs=NB) as yp,
            tc.tile_pool(name="sp", bufs=1) as sp,
            tc.tile_pool(name="up", bufs=3) as up,
            tc.tile_pool(name="op", bufs=2) as op_pool,
        ):
            # weights/params on the scalar HWDGE queue so the first x load
            # owns the sync queue from t=0
            wt = wp.tile([128, 16 * 256], FP8)
            nc.scalar.dma_start(wt[:], wts[:])
            gbt = wp.tile([128, 2], mybir.dt.float32)
            nc.scalar.dma_start(gbt[:], gb[:])

            # trigger the Prelu ACT table load during the DMA ramp (the set
            # also serves Relu)
            warm = wp.tile([128, 1], mybir.dt.float32)
            nc.vector.memset(warm[:], 1.0)
            nc.scalar.activation(warm[:], warm[:], AF.Prelu, alpha=alpha)

            # ---- AllReduce setup, off the critical path ----
            # Pay the gpsimd ext-isa IRAM load (~6us) and descriptor
            # generation for the stats broadcast now, while the conv runs.
            # The broadcast DMA itself is only triggered after the stats are
            # final (the descriptor holds addresses, not data).
            loc = sp.tile([128, 2], mybir.dt.float32, name="loc", tag="loc")
            xbuf = sp.tile([128, 16], mybir.dt.float32, name="xbuf", tag="xbuf")
            rsem = nc.alloc_semaphore("ar_remote")
            psem = nc.alloc_semaphore("ar_prep")
            lsem = nc.alloc_semaphore("ar_local")
            _AR_STATE.update(loc=loc, xbuf=xbuf, rsem=rsem,
                             psem=psem, lsem=lsem)

            _build_pass(nc, tc, xs, out, wt, gbt, alpha,
                        xp, tp, pp, ep, fp, yp, sp, up, op_pool)

    nc.compile()
    nc.m.name = f"bk{uuid.uuid4().hex[:10]}"
    return nc


def _conv_mms(nc, wt, T, ps_parts, conv_i, parity, enc):
    """Issue the 8 DoubleRow matmuls of one conv (O or E) for one batch:
    j in {0,1} accumulated over 4 psum banks. ps_parts lists
    (tile, col_offset, global_quarter) spans covering the 2048 outputs."""
    for j in range(2):
        off = (enc * 8 + parity * 4 + conv_i * 2 + j) * 256
        lhsT = wt[:, off:off + 256].rearrange("p (two m) -> p two m", two=2)
        for tile_, base_col, q in ps_parts:
            rhs = T[:, 4 * j + 1024 * q: 4 * j + 1024 * q + 1024].rearrange(
                "p (n two) -> p two n", two=2)
            nc.tensor.matmul(
                tile_[:, base_col:base_col + 512], lhsT, rhs,
                start=(j == 0), stop=(j == 1),
                perf_mode=mybir.MatmulPerfMode.DoubleRow)


def _build_pass(nc, tc, xs, out, wt, gbt, alpha,
                xp, tp, pp, ep, fp, yp, sp, up, op_pool):
    # stats: cols 0:16 per-batch sum(y') (STT max accum), 16:32 sum(y'^2)
    stats = sp.tile([128, 32], mybir.dt.float32, name="stats", tag="stats")

    y_tiles = []

    def produce_pair(bp):
        """Load x for batch pair bp and build both h' tiles (fp8, shifted).
        One pair-wide is_ge (both batches in one DVE op) lands in U; DMA
        copies then place the direct and shifted-by-2 halves per batch."""
        xt = xp.tile([128, L], mybir.dt.float32, name=f"xt{bp}", tag="xt")
        nc.sync.dma_start(xt[:], xs[bp * 128:(bp + 1) * 128, :])
        U = up.tile([128, L], FP8, name=f"U{bp}", tag="U")
        enc = 0
        if True:
            # h' = (x>=0) - 0.5 on the vector engine
            nc.vector.tensor_scalar(U[:], xt[:], 0.0, 0.5,
                                    op0=ALU.is_ge, op1=ALU.subtract)
        else:
            # sign(x) = +-1 on the scalar engine (paired with 0.5x weights);
            # odd pairs always land in T buffers 2,3 so the -1.0 pads stay
            # consistent across buffer reuse
            nc.scalar.activation(U[:], xt[:], AF.Sign)
        pad = -0.5 if enc == 0 else -1.0
        Ts = []
        for sub in range(2):
            b = 2 * bp + sub
            lo, hi = (0, 64) if sub == 0 else (64, 128)
            ol, oh = (64, 128) if sub == 0 else (0, 64)
            T = tp.tile([128, TW], FP8, name=f"T{b}", tag="T")
            if b < T_BUFS:
                # first use of each of the 4 T buffers: set the pads (enc is
                # stable per buffer: even pairs -> bufs 0/1, odd -> 2/3)
                nc.vector.memset(T[lo:hi, 0:3], pad)
                nc.vector.memset(T[lo:hi, L + 3:TW], pad)
                nc.vector.memset(T[ol:oh, 0:1], pad)
                nc.vector.memset(T[ol:oh, L + 1:TW], pad)
            # direct half on sync, shifted-by-2 half on scalar (pair 0:
            # both on sync, clear of the weight DMAs on the scalar queue)
            ceng = nc.sync if bp == 0 else nc.scalar
            nc.sync.dma_start(T[lo:hi, 3:L + 3], U[lo:hi, :])
            ceng.dma_start(T[ol:oh, 1:L + 1], U[lo:hi, :])
            Ts.append(T)
        return Ts

    def conv_pair(bp, Ts):
        enc = 0
        for sub in range(2):
            b = 2 * bp + sub
            T = Ts[sub]
            # E (the last-finishing conv) in two independent half-tiles:
            # its prelu is the binding PSUM release for the next batch's
            # E-matmuls, and per-half tiles let pe start at 14/16 of the
            # batch instead of 16/16 and release banks per half
            o_full = pp.tile([128, LO], mybir.dt.float32, name=f"o{b}",
                             tag="o", bufs=1)
            e_h = [pp.tile([128, LO // 2], mybir.dt.float32,
                           name=f"e{b}_{h}", tag="eh") for h in range(2)]

            o_parts = [(o_full, 512 * q, q) for q in range(4)]
            e_parts = [(e_h[0], 0, 0), (e_h[0], 512, 1),
                       (e_h[1], 0, 2), (e_h[1], 512, 3)]
            # O first: its prelu frees banks while E still computes
            _conv_mms(nc, wt, T, o_parts, 0, sub, enc)
            _conv_mms(nc, wt, T, e_parts, 1, sub, enc)

            po = fp.tile([128, LO], mybir.dt.float16, name=f"po{b}", tag="po")
            pe = ep.tile([128, LO], mybir.dt.float16, name=f"pe{b}", tag="pe")
            nc.scalar.activation(po[:], o_full[:], AF.Prelu, alpha=alpha)
            for h in range(2):
                nc.scalar.activation(pe[:, h * 1024:(h + 1) * 1024],
                                     e_h[h][:], AF.Prelu, alpha=alpha)

            yt = yp.tile([128, LO], mybir.dt.float16, name=f"yt{b}", tag="yt")
            y_tiles.append(yt)
            # tensor_tensor has a 2x uop (STT is 1x-only); the per-channel
            # sums come from in-place 4x tensor_scalar copies with accum_out
            nc.vector.scalar_tensor_tensor(
                yt[:], pe[:], 0.0, po[:], op0=ALU.bypass, op1=ALU.max,
                accum_out=stats[:, b:b + 1])
            sq = fp.tile([128, LO], mybir.dt.float16, name=f"sq{b}", tag="po")
            nc.vector.scalar_tensor_tensor(
                sq[:], yt[:], 1.0, yt[:], op0=ALU.mult, op1=ALU.mult,
                accum_out=stats[:, 16 + b:17 + b])

    Ts_ahead = produce_pair(0)
    for bp in range(NB // 2):
        Ts_cur = Ts_ahead
        if bp + 1 < NB // 2:
            Ts_ahead = produce_pair(bp + 1)
        conv_pair(bp, Ts_cur)

    # ---- local partial stats -> remote-DMA all-reduce -> scale/shift ----
    # One-shot broadcast: every core sends its [128,2] partial (sum, sumsq)
    # to ALL 8 cores (itself included) in a single remote_dma_broadcast --
    # the ucode fans the 8 destinations across all 16 DMA engines in
    # parallel, vs ~7 serialized CC transfers for per-peer sends. The
    # landing slot is indexed by the sender id (runtime partition_id offset)
    # so every slot is single-writer. The descriptor was pre-generated at
    # kernel start (see above); here we only barrier + trigger + wait.
    loc = _AR_STATE["loc"]
    nc.vector.tensor_reduce(loc[:, 0:1], stats[:, 0:16],
                            axis=mybir.AxisListType.X, op=ALU.add)
    nc.vector.tensor_reduce(loc[:, 1:2], stats[:, 16:32],
                            axis=mybir.AxisListType.X, op=ALU.add)

    xbuf = _AR_STATE["xbuf"]
    g = sp.tile([128, 2], mybir.dt.float32, name="g", tag="g")

    rsem = _AR_STATE["rsem"]
    psem = _AR_STATE["psem"]
    lsem = _AR_STATE["lsem"]
    with tc.tile_critical(no_gpsimd_drain=True):
        nc.gpsimd.bir_kernel_barrier_wait([list(range(N_CORES))])
        # slot = this core's id, so every slot is single-writer
        slot = xbuf[:, 0:2].copy()
        slot.offset = nc.gpsimd.partition_id() * 2
        nc.gpsimd.remote_dma_broadcast(
            slot, loc[:, 0:2], rsem, lsem,
            rdests=[(0, k) for k in range(N_CORES)],
        ).then_inc(psem, 1)
        nc.gpsimd.wait_ge(psem, 1)
        nc.gpsimd.trigger_dma(count=None)
        # 8 arriving broadcasts (incl self) x (16//8)=2 incs each
        nc.vector.wait_ge(rsem, 16)
        nc.vector.tensor_reduce(
            g[:, 0:1], xbuf.rearrange("p (s two) -> p two s", two=2)[:, 0:1, :],
            axis=mybir.AxisListType.X, op=ALU.add)
        nc.vector.tensor_reduce(
            g[:, 1:2], xbuf.rearrange("p (s two) -> p two s", two=2)[:, 1:2, :],
            axis=mybir.AxisListType.X, op=ALU.add)

    # mean/var/scale/shift, all [128,1] f32. y = 2*y' so:
    #   mean = 2*S1/M ; E[y^2] = 4*S2/M ; out = (2*s)*y' + (beta - s*mean)
    v = sp.tile([128, 8], mybir.dt.float32, name="v", tag="v")
    mean, msq_eps, vareps, std, rec, t1, s2_col, t_col = (
        v[:, i:i + 1] for i in range(8))
    nc.vector.tensor_scalar(mean, g[:, 0:1], 2.0 / M_GLOBAL, None, op0=ALU.mult)
    # msq_eps = mean^2 - eps
    nc.vector.tensor_scalar(msq_eps, mean, mean, EPS, op0=ALU.mult, op1=ALU.subtract)
    # vareps = 4*ssq/M - (mean^2 - eps) = var + eps
    nc.vector.scalar_tensor_tensor(
        vareps, g[:, 1:2], 4.0 / M_GLOBAL, msq_eps,
        op0=ALU.mult, op1=ALU.subtract)
    nc.scalar.activation(std, vareps, AF.Sqrt)
    # one Newton step: std = 0.5*(std + vareps/std)
    nc.vector.reciprocal(rec, std)
    # t1 = 0.5 * vareps / std
    nc.vector.tensor_scalar(t1, rec, vareps, 0.5, op0=ALU.mult, op1=ALU.mult)
    nc.vector.scalar_tensor_tensor(std, std, 0.5, t1,
                                   op0=ALU.mult, op1=ALU.add)
    nc.vector.reciprocal(rec, std)
    # s2 = 2 * gamma / std
    nc.vector.tensor_scalar(s2_col, rec, gbt[:, 0:1], 2.0,
                            op0=ALU.mult, op1=ALU.mult)
    # t = beta - s2*mean/2
    nc.vector.tensor_scalar(t1, mean, -0.5, None, op0=ALU.mult)
    nc.vector.scalar_tensor_tensor(
        t_col, s2_col, t1, gbt[:, 1:2], op0=ALU.mult, op1=ALU.add)

    # ---- pass 2: normalize + store. Two batches per output tile (fewer,
    # bigger DMAs); out-DMAs alternate across both HWDGE queues ----
    for bp in range(NB // 2):
        ot = op_pool.tile([128, 2 * LO], mybir.dt.float32, name=f"ot{bp}", tag="ot")
        for sub in range(2):
            nc.scalar.activation(
                ot[:, sub * LO:(sub + 1) * LO], y_tiles[2 * bp + sub][:],
                AF.Identity, bias=t_col, scale=s2_col)
        eng = nc.sync if bp % 2 == 0 else nc.scalar
        eng.dma_start(
            out.rearrange("(a p) l -> p a l", p=128)[:, 2 * bp:2 * bp + 2, :],
            ot.rearrange("p (a l) -> p a l", a=2))


def _swinterleave(wt):
    """Repack [128, nsets, 2, 128] lhsT sets into the DoubleRowSwInterleave
    layout: per partition row [A127, B127, A126, B126, ..., A0, B0]."""
    k, ns, two, m = wt.shape
    out = np.zeros((k, ns, 2 * m), wt.dtype)
    for c in range(m):
        out[:, :, 2 * c] = wt[:, :, 0, m - 1 - c]
        out[:, :, 2 * c + 1] = wt[:, :, 1, m - 1 - c]
    return out.reshape(k, ns, 2, m)


def _prep_weights(W: np.ndarray) -> np.ndarray:
    """Host-side: pack the 8 DoubleRow lhsT matrices [128, 2, 128] fp8:
    (parity even/odd) x (conv O/E) x (j 0/1). Partition k=(c,g): channel
    c=k%64, shift s(k) (0/2 direct/shifted, swapped for odd parity); pair
    element i is tap 4j + i + s(k)."""
    bw = np.sign(W).astype(np.float32)          # [CO, CI, K]
    wh = np.zeros((CO, CI, 8), np.float32)
    wh[:, :, :K] = bw
    wo = np.zeros((CO, CI, 8), np.float32)      # O-conv taps: w[t-1]
    wo[:, :, 1:8] = wh[:, :, 0:7]

    wt = np.zeros((128, 8, 2, 128), np.float32)  # [k, set, i, o]
    ks = np.arange(128)
    cs = ks % 64
    for parity in range(2):
        s_of_k = np.where(ks < 64, 0, 2) if parity == 0 else \
            np.where(ks < 64, 2, 0)
        for conv_i, wsrc in enumerate([wo, wh]):
            for j in range(2):
                si = (parity * 2 + conv_i) * 2 + j
                for i in range(2):
                    t = 4 * j + i + s_of_k          # [128]
                    wt[ks, si, i, :] = wsrc[:, cs, t].T
    # enc 1 (activations +-1 via ACT Sign) uses the same weights halved,
    # so both encodings produce identical conv values
    full = np.concatenate([wt, 0.5 * wt], axis=1)  # [128, 16, 2, 128]
    return full.reshape(128, 16 * 256).astype(FP8NP)


_NC_CACHE = {}


def kernel(x, W, prelu_w, gamma, beta):
    x = np.asarray(x)
    W = np.asarray(W)
    alpha = float(np.asarray(prelu_w).reshape(-1)[0])
    gamma = np.asarray(gamma, dtype=np.float32)
    beta = np.asarray(beta, dtype=np.float32)

    assert x.shape == (N, CI, L), x.shape
    wts = _prep_weights(W)
    gb = np.stack([gamma, beta], axis=1).astype(np.float32)

    key = alpha
    if key not in _NC_CACHE:
        _NC_CACHE[key] = _build(alpha)
    nc = _NC_CACHE[key]

    in_maps = []
    for c in range(N_CORES):
        shard = np.ascontiguousarray(
            x[c * NB:(c + 1) * NB].reshape(NB * CI, L), dtype=np.float32)
        in_maps.append({"xs": shard, "wts": wts, "gb": gb})

    res = run_bass_kernel_spmd(nc, in_maps, core_ids=list(range(N_CORES)))
    outs = [res.results[c]["out"].reshape(NB, CO, LO) for c in range(N_CORES)]
    return np.concatenate(outs, axis=0)
